# revision 50
# baseline (speedup 1.0000x reference)
"""MoE expert-parallel kernel for Trainium2 (8 NeuronCores).

Problem: nn_DistributedExpertPool — each of 2048 tokens (H=1024) is routed to
one of 8 experts; expert e applies Linear(H->F=2048) -> exact GELU ->
Linear(F->H).

Strategy (expert parallelism, matching the sharding hint):
  - Host: sort tokens by expert assignment ("dispatch"), pad each expert's
    token batch to a common capacity CAP, and pre-transpose to x.T layout so
    the device kernel only ever streams K-major operands.
  - Core c gets expert c's weights plus its token batch, computes
    y.T = W2.T @ gelu(W1.T @ x.T + b1) + b2 entirely on-chip.
  - Host: scatter each core's outputs back to the original token order
    ("combine").

Schedule notes (all sizes for cap~274):
  - Phase 1 opens with a k-outer sweep over output chains 0..7 so the first
    matmuls only need chunk0 = [x_k0 | W1 k-row 0 of chains 0..7] (~330KB)
    instead of all of x + a full W1 strip (~820KB). Chunks k=1..7 stream one
    per k-step, each arriving just ahead of the PE.
  - Chains 8..15 then run chain-outer against per-chain W1 strips (their PSUM
    banks free up as the group-A GELUs drain).
  - Phase 2 is chain-outer over 8 H-chains against a k-major W2 stream; the
    Tile scheduler interleaves chains so the PE never waits on a W2 quad.
  - The last H-chain is split column-wise so the final store is a narrow
    [128,128] tile — shortens the post-matmul tail.
  - b1/b2 ride as extra fp16-bitcast columns on chunk7 (no extra DMA).
  - Evacs/stores are fp16 (output rounding ~5e-4 relative, tolerance 2e-2).

Matmul operands stream as fp16 (weights ~N(0, 0.02), activations O(1) — well
inside fp16 range). PSUM accumulation stays fp32. Measured ~4e-4 relative
error end-to-end vs the fp32 reference.
"""

import os as _os
import sys as _sys

import numpy as np

try:
    import concourse.bass as bass
except ImportError:  # fresh dirs without the site hook on sys.path
    for _p in ("/opt/trn_rl_repo", "/root/.axon_site/_ro/trn_rl_repo"):
        if _p not in _sys.path:
            _sys.path.append(_p)
    import concourse.bass as bass  # noqa: E402
import concourse.tile as tile
from concourse import mybir
from concourse.bass_utils import run_bass_kernel_spmd  # noqa: F401 (fallback)

_jit_cache: dict[int, tuple] = {}


def _run_spmd_cached(nc, in_maps):
    """run_bass_kernel_spmd's axon/PJRT path with the jitted executable cached
    per program — the concourse shim rebuilds its jax.jit closure every call,
    paying ~1.5s of retrace; reusing one function object makes repeat calls
    dispatch in milliseconds."""
    import jax
    import numpy as _np
    from jax.sharding import Mesh, PartitionSpec
    from jax.experimental.shard_map import shard_map
    from concourse import bass2jax, mybir as _mb

    key = id(nc)
    if key not in _jit_cache:
        bass2jax.install_neuronx_cc_hook()
        partition_name = (nc.partition_id_tensor.name
                          if nc.partition_id_tensor else None)
        in_names, out_names, out_avals = [], [], []
        for alloc in nc.m.functions[0].allocations:
            if not isinstance(alloc, _mb.MemoryLocationSet):
                continue
            name = alloc.memorylocations[0].name
            if alloc.kind == "ExternalInput":
                if name != partition_name:
                    in_names.append(name)
            elif alloc.kind == "ExternalOutput":
                out_names.append(name)
                out_avals.append(jax.core.ShapedArray(
                    tuple(alloc.tensor_shape), _mb.dt.np(alloc.dtype)))
        n_params = len(in_names)
        all_names = list(in_names) + list(out_names)
        if partition_name is not None:
            all_names.append(partition_name)

        def _body(*args):
            operands = list(args)
            if partition_name is not None:
                operands.append(bass2jax.partition_id_tensor())
            return tuple(bass2jax._bass_exec_p.bind(
                *operands, out_avals=tuple(out_avals),
                in_names=tuple(all_names), out_names=tuple(out_names),
                lowering_input_output_aliases=(),
                sim_require_finite=True, sim_require_nnan=True, nc=nc))

        devices = jax.devices()[:N_CORES]
        mesh = Mesh(_np.asarray(devices), ("core",))
        n_outs = len(out_names)
        sharded = jax.jit(
            shard_map(_body, mesh=mesh,
                      in_specs=(PartitionSpec("core"),) * (n_params + n_outs),
                      out_specs=(PartitionSpec("core"),) * n_outs,
                      check_rep=False),
            donate_argnums=tuple(range(n_params, n_params + n_outs)),
            keep_unused=True)
        _jit_cache[key] = (sharded, in_names, out_names, out_avals, n_params)

    sharded, in_names, out_names, out_avals, n_params = _jit_cache[key]
    concat_in = [
        _np.concatenate([_np.asarray(m[name]) for m in in_maps], axis=0)
        for name in in_names]
    concat_zeros = [
        _np.zeros((N_CORES * a.shape[0], *a.shape[1:]), a.dtype)
        for a in out_avals]
    out_arrs = sharded(*concat_in, *concat_zeros)

    class _R:
        results = [
            {name: _np.asarray(out_arrs[i]).reshape(
                N_CORES, *out_avals[i].shape)[c]
             for i, name in enumerate(out_names)}
            for c in range(N_CORES)]
    return _R()

TOKENS = 2048
HIDDEN = 1024
FFN = 2048
NUM_EXPERTS = 8
N_CORES = 8

KH = HIDDEN // 128  # 8 K-tiles for the first matmul
KF = FFN // 128     # 16 K-tiles for the second matmul

_compiled_cache: dict[tuple, bass.Bass] = {}

MM_DTYPE = {"fp32": mybir.dt.float32, "fp32r": mybir.dt.float32r,
            "fp16": mybir.dt.float16}[_os.environ.get("KM_MMDT", "fp16")]
N_WARM = int(_os.environ.get("KM_WARM", "26"))


def _split_multi_waits(nc: bass.Bass) -> None:
    """Walrus in this toolchain accepts at most ONE sync-wait per instruction
    ("Too many sync wait commands" in setupSyncWait otherwise). Tile's
    scheduler happily attaches several. Split the extras into NoOps placed
    just before the instruction on the same engine queue — the NX sequencer
    processes them in order, so the semantics are identical."""
    for fn in nc.m.functions:
        for blk in fn.blocks:
            out = []
            changed = False
            for inst in blk.instructions:
                si = inst.sync_info
                if si is not None and si.on_wait is not None and len(si.on_wait) > 1:
                    # DMA-lane sems satisfy last (data + 900ns prop): putting
                    # them at the tail keeps the already-satisfied engine-sem
                    # NoOps off the critical path of the final drain.
                    waits = sorted(
                        si.on_wait,
                        key=lambda w: str(w.ant_name or "").startswith(
                            ("DMAHW", "DMASW")))
                    for j, w in enumerate(waits[:-1]):
                        nop = mybir.InstNoOp(
                            name=f"{inst.name}-wsplit{j}", ins=[], outs=[])
                        nop.engine = inst.engine
                        nop.sync_info = mybir.SyncInfo(on_wait=[w], on_update=[])
                        out.append(nop)
                    inst.sync_info = mybir.SyncInfo(
                        on_wait=[waits[-1]],
                        on_update=list(si.on_update) if si.on_update else [],
                    )
                    changed = True
                out.append(inst)
            if changed:
                blk.instructions = out


def _wire_prep_dma_sems(nc: bass.Bass) -> None:
    """Point each PREPARE_ONLY prep's DMA-completion sem (on_update[0]) at the
    DMASW lane semaphore Tile's epilogue actually waits on. Tile attributes
    the deferred DMA write to the prep's DMASW tick, but leaves the
    user-supplied sem= in on_update[0] — so the lane sem would never move and
    the end-of-kernel drain deadlocks. Rewiring the update target makes the
    descriptor-baked sem BE the lane sem."""
    for fn in nc.m.functions:
        updated: set[int] = set()
        waited: dict[int, str] = {}
        insts = [i for blk in fn.blocks for i in blk.instructions]
        for inst in insts:
            si = inst.sync_info
            if si is None:
                continue
            for u in (si.on_update or []):
                if u.sync_type == "semaphore":
                    updated.add(u.id)
            for w in (si.on_wait or []):
                if w.sync_type == "semaphore" and str(w.ant_name or "").startswith("DMASW"):
                    waited[w.id] = w.ant_name
        dangling = [(i, n) for i, n in sorted(waited.items()) if i not in updated]
        if not dangling:
            continue
        # Tile assigns DMASW lanes to Pool-engine DMA instructions round-robin
        # in program order; mirror that to pair each prep with its lane.
        lanes = sorted(dangling, key=lambda t: t[1])
        preps = [i for i in insts if getattr(i, "gen_mode", 0) == 1]
        for pi, prep in enumerate(preps):
            sem_id, sem_name = lanes[pi % len(lanes)]
            u0 = prep.sync_info.on_update[0]
            u0.id = sem_id
            u0.ant_name = sem_name


def _build_nc(cap: int, mm_dtype=None) -> bass.Bass:
    """Build the per-core Bass program for token capacity `cap` (mult of 2)."""
    fp32 = mybir.dt.float32
    mmdt = MM_DTYPE if mm_dtype is None else mm_dtype
    nc = bass.Bass("TRN2", target_bir_lowering=False, debug=False,
                   num_devices=N_CORES)

    CK = cap + KH * 128            # chunk width: x k-block + W1 k-row of A
    # biases ride on chunk7 as raw lanes of the stream dtype; fp32 biases
    # need 2 fp16 lanes per value
    LW = 4 // mybir.dt.size(mmdt)  # lanes per fp32 value
    BIAS = LW * (KF + KH)
    # xw: 8 chunks (last one carries biases) + strips for chains 8..15
    xw_d = nc.dram_tensor("xw", [128, KH * CK + BIAS + KH * 1024], mmdt,
                          kind="ExternalInput").ap()
    # w2s: k-major quads [k-slice 0..3 | 4..7 | ...], k-slice = [128, HIDDEN]
    w2s_d = nc.dram_tensor("w2s", [128, KF * HIDDEN], mmdt,
                           kind="ExternalInput").ap()
    y16 = nc.dram_tensor("y16", [HIDDEN, cap], mmdt, kind="ExternalOutput").ap()

    SPL = cap - 128  # last two H-chains split into [0:SPL) and [SPL:cap)

    with tile.TileContext(nc) as tc:
        with (
            tc.tile_pool(name="ck_pool", bufs=KH) as ck_pool,
            tc.tile_pool(name="w1_pool", bufs=KH) as w1_pool,
            tc.tile_pool(name="w2_pool", bufs=1) as w2_pool,
            tc.tile_pool(name="wz_pool", bufs=1) as wz_pool,
            tc.tile_pool(name="ht_pool", bufs=KF) as ht_pool,
            tc.tile_pool(name="out_pool", bufs=4) as out_pool,
            tc.tile_pool(name="ps_pool", bufs=8, space="PSUM") as ps_pool,
        ):
            # --- load stream (SP queue): chunks 0..7, strips, quads ---
            # chunk0 splits into [x_k0 | W1 m0-tile] + [W1 m1..7]: the first
            # matmul's operands land one small DMA earlier, buying ~2 extra
            # matmuls inside the PE's half-clock ramp window
            ck0a = ck_pool.tile([128, cap + 128], mmdt, name="ck0a",
                                tag="ck0a", bufs=1)
            nc.sync.dma_start(ck0a[:], xw_d[:, :cap + 128])
            ck0b = ck_pool.tile([128, 7 * 128], mmdt, name="ck0b",
                                tag="ck0b", bufs=1)
            nc.sync.dma_start(ck0b[:], xw_d[:, cap + 128:CK])
            cks = [ck0a]
            for k in range(1, KH):
                w = CK + (BIAS if k == KH - 1 else 0)
                t = ck_pool.tile([128, w], mmdt, name=f"ck{k}", tag=f"ck{k}",
                                 bufs=1)
                off = k * CK
                nc.sync.dma_start(t[:], xw_d[:, off:off + w])
                cks.append(t)
            b1s = cks[7][:, CK:CK + LW * KF].bitcast(fp32)      # [128, KF]
            b2s = cks[7][:, CK + LW * KF:CK + BIAS].bitcast(fp32)  # [128, KH]

            def w1a_lhsT(k, m):
                if k == 0:
                    return (ck0a[:, cap:cap + 128] if m == 0
                            else ck0b[:, (m - 1) * 128:m * 128])
                return cks[k][:, cap + m * 128:cap + (m + 1) * 128]

            sps = []
            for j in range(KH):
                t = w1_pool.tile([128, 1024], mmdt, name=f"sp{j}",
                                 tag=f"sp{j}", bufs=1)
                off = KH * CK + BIAS + j * 1024
                nc.sync.dma_start(t[:], xw_d[:, off:off + 1024])
                sps.append(t)

            # quad0 streams as four k-slice DMAs so phase 2's first wave isn't
            # gated on the whole 1MB quad; quads 1..3 stay whole.
            w2qs = []
            q0 = w2_pool.tile([128, 4 * HIDDEN], mmdt, name="w2q0",
                              tag="w2q0", bufs=1)
            for kk in range(4):
                nc.sync.dma_start(q0[:, kk * HIDDEN:(kk + 1) * HIDDEN],
                                  w2s_d[:, kk * HIDDEN:(kk + 1) * HIDDEN])
            w2qs.append(q0)
            for q in range(1, 4):
                t = w2_pool.tile([128, 4 * HIDDEN], mmdt, name=f"w2q{q}",
                                 tag=f"w2q{q}", bufs=1)
                nc.sync.dma_start(
                    t[:], w2s_d[:, q * 4 * HIDDEN:(q + 1) * 4 * HIDDEN])
                w2qs.append(t)

            # PE p-state warmup: the PE runs below max clock until ~3us of
            # continuous busy. Burn the DMA-wait window with tiny matmuls on
            # a zeroed tile so the real stream starts warm.
            wz = wz_pool.tile([128, 128], mmdt, name="wz", tag="wz")
            nc.gpsimd.memset(wz[:], 0.0)
            wps = ps_pool.tile([128, 128], fp32, name="wps", tag="ps")
            for i in range(N_WARM):
                nc.tensor.matmul(wps[:], wz[:], wz[:], start=True, stop=True)

            o6b = out_pool.tile([128, 128], mmdt, name="os6b", tag="os6b")
            o7b = out_pool.tile([128, 128], mmdt, name="os7b", tag="os7b")

            # ---- phase 1, group A (chains 0..7): k-outer sweep ----
            hts = [None] * KF
            psA = [ps_pool.tile([128, cap], fp32, name=f"ps1_{m}", tag="ps")
                   for m in range(KH)]
            for k in range(KH):
                xk = cks[k][:, :cap]
                for m in range(KH):
                    nc.tensor.matmul(
                        psA[m][:], w1a_lhsT(k, m),
                        xk, start=(k == 0), stop=(k == KH - 1))
            for m in range(KH):
                ht = ht_pool.tile([128, cap], mmdt, name=f"ht{m}", tag="ht")
                nc.scalar.activation(
                    ht[:], psA[m][:], mybir.ActivationFunctionType.Gelu,
                    bias=b1s[:, m:m + 1])
                hts[m] = ht

            # ---- phase 1, chains 8..15: chain-outer against W1 strips ----
            for m in range(KH, KF):
                psb = ps_pool.tile([128, cap], fp32, name=f"ps1_{m}", tag="ps")
                for k in range(KH):
                    lhsT = sps[m - KH][:, k * 128:(k + 1) * 128]
                    nc.tensor.matmul(psb[:], lhsT, cks[k][:, :cap],
                                     start=(k == 0), stop=(k == KH - 1))
                ht = ht_pool.tile([128, cap], mmdt, name=f"ht{m}", tag="ht")
                nc.scalar.activation(
                    ht[:], psb[:], mybir.ActivationFunctionType.Gelu,
                    bias=b1s[:, m:m + 1])
                hts[m] = ht

            # ---- phase 2: yT[m] = W2.T @ hT + b2  [H on partitions] ----
            def w2_lhsT(k, m):
                q, kk = divmod(k, 4)
                off = kk * HIDDEN + m * 128
                return w2qs[q][:, off:off + 128]

            # chain order: full chains 0..5 first (their stops happen right
            # after the last W2 quad lands, so their pair stores hide under
            # the remaining chains), then the [0:SPL) halves of chains 7 and
            # 6 (one pair store), then the [SPL:cap) halves of 6 and 7 LAST,
            # exiting through the prepared scatters above — the post-last-
            # matmul tail is just evac + trigger + transfer + sem.
            # chains 0..3: one quad store (rows 0..511) fired on m3's evac
            ot4 = out_pool.tile([128, 4 * cap], mmdt, name="ot4", tag="ot4")
            for m in range(4):
                ps2 = ps_pool.tile([128, cap], fp32, name=f"ps2_{m}", tag="ps")
                for k in range(KF):
                    nc.tensor.matmul(ps2[:], w2_lhsT(k, m), hts[k][:],
                                     start=(k == 0), stop=(k == KF - 1))
                blk = ot4[:, m * cap:(m + 1) * cap]
                if m % 2 == 0:
                    nc.vector.tensor_scalar_add(blk, ps2[:], b2s[:, m:m + 1])
                else:
                    nc.scalar.activation(
                        blk, ps2[:], mybir.ActivationFunctionType.Identity,
                        bias=b2s[:, m:m + 1])
            nc.sync.dma_start(
                y16[0:512, :].rearrange("(c p) t -> p c t", p=128),
                ot4.rearrange("p (c t) -> p c t", c=4))

            # chains 4,5: pair store on the ACT ring
            ot2 = out_pool.tile([128, 2 * cap], mmdt, name="ot2", tag="ot2")
            for m in (4, 5):
                ps2 = ps_pool.tile([128, cap], fp32, name=f"ps2_{m}", tag="ps")
                for k in range(KF):
                    nc.tensor.matmul(ps2[:], w2_lhsT(k, m), hts[k][:],
                                     start=(k == 0), stop=(k == KF - 1))
                blk = ot2[:, (m - 4) * cap:(m - 3) * cap]
                if m % 2 == 0:
                    nc.vector.tensor_scalar_add(blk, ps2[:], b2s[:, m:m + 1])
                else:
                    nc.scalar.activation(
                        blk, ps2[:], mybir.ActivationFunctionType.Identity,
                        bias=b2s[:, m:m + 1])
            nc.scalar.dma_start(
                y16[512:768, :].rearrange("(c p) t -> p c t", p=128),
                ot2.rearrange("p (c t) -> p c t", c=2))

            # [0:SPL) halves of chains 7 then 6, one paired store on SP
            ota = out_pool.tile([128, 2 * SPL], mmdt, name="ota", tag="ota")
            for i, m in enumerate((KH - 1, KH - 2)):
                psx = ps_pool.tile([128, SPL], fp32, name=f"ps2_{m}a",
                                   tag="ps")
                for k in range(KF):
                    nc.tensor.matmul(psx[:], w2_lhsT(k, m), hts[k][:, :SPL],
                                     start=(k == 0), stop=(k == KF - 1))
                if i == 0:  # chain 7 half -> second column block
                    nc.scalar.activation(
                        ota[:, SPL:], psx[:],
                        mybir.ActivationFunctionType.Identity,
                        bias=b2s[:, m:m + 1])
                else:       # chain 6 half -> first column block, then store
                    nc.vector.tensor_scalar_add(ota[:, :SPL], psx[:],
                                                b2s[:, m:m + 1])
                    nc.sync.dma_start(
                        y16[(KH - 2) * 128:KH * 128, :SPL]
                        .rearrange("(c p) t -> p c t", p=128),
                        ota.rearrange("p (c t) -> p c t", c=2))

            # [SPL:cap) halves of chains 6 then 7 run LAST — narrow stores on
            # separate queues keep the post-last-matmul tail short
            for m, ob in ((KH - 2, o6b), (KH - 1, o7b)):
                psx = ps_pool.tile([128, 128], fp32, name=f"ps2_{m}b",
                                   tag="ps")
                for k in range(KF):
                    nc.tensor.matmul(psx[:], w2_lhsT(k, m), hts[k][:, SPL:],
                                     start=(k == 0), stop=(k == KF - 1))
                nc.vector.tensor_scalar_add(ob[:], psx[:], b2s[:, m:m + 1])
                eng = nc.scalar if m == KH - 2 else nc.sync
                eng.dma_start(y16[m * 128:(m + 1) * 128, SPL:], ob[:])

    _wire_prep_dma_sems(nc)
    _split_multi_waits(nc)
    return nc


def _get_nc(cap: int) -> bass.Bass:
    key = (cap, MM_DTYPE, N_WARM)
    if key not in _compiled_cache:
        _compiled_cache[key] = _build_nc(cap, MM_DTYPE)
    return _compiled_cache[key]


def _reference_numpy(x, idx, W1, b1, W2, b2):
    """Exact CPU path (erf-gelu in float64). Used only if routing is so
    imbalanced that one expert exceeds 512 tokens (breaks the device tiling)
    or the device path fails — slow but correct."""
    import math
    erf = np.vectorize(math.erf, otypes=[np.float64])
    out = np.zeros_like(x, dtype=np.float64)
    for e in range(NUM_EXPERTS):
        rows = np.nonzero(idx == e)[0]
        if rows.size == 0:
            continue
        h = x[rows].astype(np.float64) @ W1[e].astype(np.float64) + b1[e]
        h = h * 0.5 * (1.0 + erf(h / np.sqrt(2.0)))
        out[rows] = h @ W2[e].astype(np.float64) + b2[e]
    return out.astype(np.float32)


def kernel(x, expert_indices, W1, b1, W2, b2):
    x = np.ascontiguousarray(np.asarray(x, dtype=np.float32))
    idx = np.asarray(expert_indices).astype(np.int64)
    W1 = np.asarray(W1, dtype=np.float32)
    W2 = np.asarray(W2, dtype=np.float32)
    b1 = np.asarray(b1, dtype=np.float32)
    b2 = np.asarray(b2, dtype=np.float32)

    counts = np.bincount(idx, minlength=NUM_EXPERTS)
    # one PSUM bank caps the per-chain moving dim at 512 fp32
    cap = max(256, int(-(-int(counts.max()) // 2)) * 2)
    if cap > 512:  # pathological routing, exceeds one PSUM bank
        return _reference_numpy(x, idx, W1, b1, W2, b2)
    nc = _get_nc(cap)

    # dispatch: stable sort tokens by expert
    order = np.argsort(idx, kind="stable")
    starts = np.zeros(NUM_EXPERTS + 1, dtype=np.int64)
    np.cumsum(counts, out=starts[1:])

    np_mmdt = np.float16 if MM_DTYPE == mybir.dt.float16 else np.float32
    CK = cap + HIDDEN
    in_maps = []
    tok_of_core = []
    for e in range(NUM_EXPERTS):
        toks = order[starts[e]:starts[e + 1]]
        tok_of_core.append(toks)
        xs = np.zeros((KH, 128, cap), dtype=np_mmdt)
        xs.reshape(HIDDEN, cap)[:, :len(toks)] = x[toks].T
        w1e = W1[e].astype(np_mmdt)
        w1rows = w1e.reshape(KH, 128, FFN)
        biasv = np.concatenate([
            np.ascontiguousarray(b1[e].reshape(KF, 128).T),
            np.ascontiguousarray(b2[e].reshape(KH, 128).T),
        ], axis=1).view(np_mmdt)  # fp32 biases as raw lanes of stream dtype
        chunks = []
        for k in range(KH):
            parts = [xs[k], w1rows[k][:, :HIDDEN]]
            if k == KH - 1:
                parts.append(biasv)
            chunks.append(np.concatenate(parts, axis=1))
        strips = w1e[:, HIDDEN:].reshape(KH, 128, KH, 128) \
            .transpose(1, 2, 0, 3).reshape(128, -1)  # [128, 8 strips of 1024]
        xw = np.concatenate(chunks + [strips], axis=1)
        w2s = W2[e].astype(np_mmdt).reshape(KF, 128, HIDDEN) \
            .transpose(1, 0, 2).reshape(128, -1)
        in_maps.append({
            "xw": np.ascontiguousarray(xw),
            "w2s": np.ascontiguousarray(w2s),
        })

    try:
        res = _run_spmd_cached(nc, in_maps)
    except Exception:
        try:  # transient failures recover on retry; fall back to the shim
            res = run_bass_kernel_spmd(nc, in_maps,
                                       core_ids=list(range(N_CORES)))
        except Exception:
            return _reference_numpy(x, idx, W1, b1, W2, b2)
    global LAST_RESULTS
    LAST_RESULTS = res

    out = np.zeros((TOKENS, HIDDEN), dtype=np.float32)
    for e in range(NUM_EXPERTS):
        toks = tok_of_core[e]
        yT = res.results[e]["y16"].astype(np.float32)
        out[toks] = yT[:, :len(toks)].T
    return out


# revision 51
# speedup vs baseline: 1.0088x; 1.0088x over previous
"""MoE expert-parallel kernel for Trainium2 (8 NeuronCores).

Problem: nn_DistributedExpertPool — each of 2048 tokens (H=1024) is routed to
one of 8 experts; expert e applies Linear(H->F=2048) -> exact GELU ->
Linear(F->H).

Strategy (expert parallelism, matching the sharding hint):
  - Host: sort tokens by expert assignment ("dispatch"), pad each expert's
    token batch to a common capacity CAP, and pre-transpose to x.T layout so
    the device kernel only ever streams K-major operands.
  - Core c gets expert c's weights plus its token batch, computes
    y.T = W2.T @ gelu(W1.T @ x.T + b1) + b2 entirely on-chip.
  - Host: scatter each core's outputs back to the original token order
    ("combine").

Schedule notes (all sizes for cap~274):
  - Phase 1 opens with a k-outer sweep over output chains 0..7 so the first
    matmuls only need chunk0 = [x_k0 | W1 k-row 0 of chains 0..7] (~330KB)
    instead of all of x + a full W1 strip (~820KB). Chunks k=1..7 stream one
    per k-step, each arriving just ahead of the PE.
  - Chains 8..15 then run chain-outer against per-chain W1 strips (their PSUM
    banks free up as the group-A GELUs drain).
  - Phase 2 is chain-outer over 8 H-chains against a k-major W2 stream; the
    Tile scheduler interleaves chains so the PE never waits on a W2 quad.
  - The last H-chain is split column-wise so the final store is a narrow
    [128,128] tile — shortens the post-matmul tail.
  - b1/b2 ride as extra fp16-bitcast columns on chunk7 (no extra DMA).
  - Evacs/stores are fp16 (output rounding ~5e-4 relative, tolerance 2e-2).

Matmul operands stream as fp16 (weights ~N(0, 0.02), activations O(1) — well
inside fp16 range). PSUM accumulation stays fp32. Measured ~4e-4 relative
error end-to-end vs the fp32 reference.
"""

import os as _os
import sys as _sys

import numpy as np

try:
    import concourse.bass as bass
except ImportError:  # fresh dirs without the site hook on sys.path
    for _p in ("/opt/trn_rl_repo", "/root/.axon_site/_ro/trn_rl_repo"):
        if _p not in _sys.path:
            _sys.path.append(_p)
    import concourse.bass as bass  # noqa: E402
import concourse.tile as tile
from concourse import mybir
from concourse.bass_utils import run_bass_kernel_spmd  # noqa: F401 (fallback)

_jit_cache: dict[int, tuple] = {}


def _run_spmd_cached(nc, in_maps):
    """run_bass_kernel_spmd's axon/PJRT path with the jitted executable cached
    per program — the concourse shim rebuilds its jax.jit closure every call,
    paying ~1.5s of retrace; reusing one function object makes repeat calls
    dispatch in milliseconds."""
    import jax
    import numpy as _np
    from jax.sharding import Mesh, PartitionSpec
    from jax.experimental.shard_map import shard_map
    from concourse import bass2jax, mybir as _mb

    key = id(nc)
    if key not in _jit_cache:
        bass2jax.install_neuronx_cc_hook()
        partition_name = (nc.partition_id_tensor.name
                          if nc.partition_id_tensor else None)
        in_names, out_names, out_avals = [], [], []
        for alloc in nc.m.functions[0].allocations:
            if not isinstance(alloc, _mb.MemoryLocationSet):
                continue
            name = alloc.memorylocations[0].name
            if alloc.kind == "ExternalInput":
                if name != partition_name:
                    in_names.append(name)
            elif alloc.kind == "ExternalOutput":
                out_names.append(name)
                out_avals.append(jax.core.ShapedArray(
                    tuple(alloc.tensor_shape), _mb.dt.np(alloc.dtype)))
        n_params = len(in_names)
        all_names = list(in_names) + list(out_names)
        if partition_name is not None:
            all_names.append(partition_name)

        def _body(*args):
            operands = list(args)
            if partition_name is not None:
                operands.append(bass2jax.partition_id_tensor())
            return tuple(bass2jax._bass_exec_p.bind(
                *operands, out_avals=tuple(out_avals),
                in_names=tuple(all_names), out_names=tuple(out_names),
                lowering_input_output_aliases=(),
                sim_require_finite=True, sim_require_nnan=True, nc=nc))

        devices = jax.devices()[:N_CORES]
        mesh = Mesh(_np.asarray(devices), ("core",))
        n_outs = len(out_names)
        sharded = jax.jit(
            shard_map(_body, mesh=mesh,
                      in_specs=(PartitionSpec("core"),) * (n_params + n_outs),
                      out_specs=(PartitionSpec("core"),) * n_outs,
                      check_rep=False),
            donate_argnums=tuple(range(n_params, n_params + n_outs)),
            keep_unused=True)
        _jit_cache[key] = (sharded, in_names, out_names, out_avals, n_params)

    sharded, in_names, out_names, out_avals, n_params = _jit_cache[key]
    concat_in = [
        _np.concatenate([_np.asarray(m[name]) for m in in_maps], axis=0)
        for name in in_names]
    concat_zeros = [
        _np.zeros((N_CORES * a.shape[0], *a.shape[1:]), a.dtype)
        for a in out_avals]
    out_arrs = sharded(*concat_in, *concat_zeros)

    class _R:
        results = [
            {name: _np.asarray(out_arrs[i]).reshape(
                N_CORES, *out_avals[i].shape)[c]
             for i, name in enumerate(out_names)}
            for c in range(N_CORES)]
    return _R()

TOKENS = 2048
HIDDEN = 1024
FFN = 2048
NUM_EXPERTS = 8
N_CORES = 8

KH = HIDDEN // 128  # 8 K-tiles for the first matmul
KF = FFN // 128     # 16 K-tiles for the second matmul

_compiled_cache: dict[tuple, bass.Bass] = {}

MM_DTYPE = {"fp32": mybir.dt.float32, "fp32r": mybir.dt.float32r,
            "fp16": mybir.dt.float16}[_os.environ.get("KM_MMDT", "fp16")]
N_WARM = int(_os.environ.get("KM_WARM", "26"))


def _split_multi_waits(nc: bass.Bass) -> None:
    """Walrus in this toolchain accepts at most ONE sync-wait per instruction
    ("Too many sync wait commands" in setupSyncWait otherwise). Tile's
    scheduler happily attaches several. Split the extras into NoOps placed
    just before the instruction on the same engine queue — the NX sequencer
    processes them in order, so the semantics are identical."""
    for fn in nc.m.functions:
        for blk in fn.blocks:
            out = []
            changed = False
            for inst in blk.instructions:
                si = inst.sync_info
                if si is not None and si.on_wait is not None and len(si.on_wait) > 1:
                    # DMA-lane sems satisfy last (data + 900ns prop): putting
                    # them at the tail keeps the already-satisfied engine-sem
                    # NoOps off the critical path of the final drain.
                    waits = sorted(
                        si.on_wait,
                        key=lambda w: str(w.ant_name or "").startswith(
                            ("DMAHW", "DMASW")))
                    for j, w in enumerate(waits[:-1]):
                        nop = mybir.InstNoOp(
                            name=f"{inst.name}-wsplit{j}", ins=[], outs=[])
                        nop.engine = inst.engine
                        nop.sync_info = mybir.SyncInfo(on_wait=[w], on_update=[])
                        out.append(nop)
                    inst.sync_info = mybir.SyncInfo(
                        on_wait=[waits[-1]],
                        on_update=list(si.on_update) if si.on_update else [],
                    )
                    changed = True
                out.append(inst)
            if changed:
                blk.instructions = out


def _wire_prep_dma_sems(nc: bass.Bass) -> None:
    """Point each PREPARE_ONLY prep's DMA-completion sem (on_update[0]) at the
    DMASW lane semaphore Tile's epilogue actually waits on. Tile attributes
    the deferred DMA write to the prep's DMASW tick, but leaves the
    user-supplied sem= in on_update[0] — so the lane sem would never move and
    the end-of-kernel drain deadlocks. Rewiring the update target makes the
    descriptor-baked sem BE the lane sem."""
    for fn in nc.m.functions:
        updated: set[int] = set()
        waited: dict[int, str] = {}
        insts = [i for blk in fn.blocks for i in blk.instructions]
        for inst in insts:
            si = inst.sync_info
            if si is None:
                continue
            for u in (si.on_update or []):
                if u.sync_type == "semaphore":
                    updated.add(u.id)
            for w in (si.on_wait or []):
                if w.sync_type == "semaphore" and str(w.ant_name or "").startswith("DMASW"):
                    waited[w.id] = w.ant_name
        dangling = [(i, n) for i, n in sorted(waited.items()) if i not in updated]
        if not dangling:
            continue
        # Tile assigns DMASW lanes to Pool-engine DMA instructions round-robin
        # in program order; mirror that to pair each prep with its lane.
        lanes = sorted(dangling, key=lambda t: t[1])
        preps = [i for i in insts if getattr(i, "gen_mode", 0) == 1]
        for pi, prep in enumerate(preps):
            sem_id, sem_name = lanes[pi % len(lanes)]
            u0 = prep.sync_info.on_update[0]
            u0.id = sem_id
            u0.ant_name = sem_name


def _build_nc(cap: int, mm_dtype=None) -> bass.Bass:
    """Build the per-core Bass program for token capacity `cap` (mult of 2)."""
    fp32 = mybir.dt.float32
    mmdt = MM_DTYPE if mm_dtype is None else mm_dtype
    nc = bass.Bass("TRN2", target_bir_lowering=False, debug=False,
                   num_devices=N_CORES)

    CK = cap + KH * 128            # chunk width: x k-block + W1 k-row of A
    # biases ride on chunk7 as raw lanes of the stream dtype; fp32 biases
    # need 2 fp16 lanes per value
    LW = 4 // mybir.dt.size(mmdt)  # lanes per fp32 value
    BIAS = LW * (KF + KH)
    # xw: 8 chunks (last one carries biases) + strips for chains 8..15
    xw_d = nc.dram_tensor("xw", [128, KH * CK + BIAS + KH * 1024], mmdt,
                          kind="ExternalInput").ap()
    # w2s: k-major quads [k-slice 0..3 | 4..7 | ...], k-slice = [128, HIDDEN]
    w2s_d = nc.dram_tensor("w2s", [128, KF * HIDDEN], mmdt,
                           kind="ExternalInput").ap()
    y16 = nc.dram_tensor("y16", [HIDDEN, cap], mmdt, kind="ExternalOutput").ap()

    SPL = cap - 128  # last two H-chains split into [0:SPL) and [SPL:cap)

    with tile.TileContext(nc) as tc:
        with (
            tc.tile_pool(name="ck_pool", bufs=KH) as ck_pool,
            tc.tile_pool(name="w1_pool", bufs=KH) as w1_pool,
            tc.tile_pool(name="w2_pool", bufs=1) as w2_pool,
            tc.tile_pool(name="wz_pool", bufs=1) as wz_pool,
            tc.tile_pool(name="ht_pool", bufs=KF) as ht_pool,
            tc.tile_pool(name="out_pool", bufs=4) as out_pool,
            tc.tile_pool(name="ps_pool", bufs=8, space="PSUM") as ps_pool,
        ):
            # --- load stream (SP queue): chunks 0..7, strips, quads ---
            cks = []
            for k in range(KH):
                w = CK + (BIAS if k == KH - 1 else 0)
                t = ck_pool.tile([128, w], mmdt, name=f"ck{k}", tag=f"ck{k}",
                                 bufs=1)
                off = k * CK
                nc.sync.dma_start(t[:], xw_d[:, off:off + w])
                cks.append(t)
            b1s = cks[7][:, CK:CK + LW * KF].bitcast(fp32)      # [128, KF]
            b2s = cks[7][:, CK + LW * KF:CK + BIAS].bitcast(fp32)  # [128, KH]

            def w1a_lhsT(k, m):
                return cks[k][:, cap + m * 128:cap + (m + 1) * 128]

            sps = []
            for j in range(KH):
                t = w1_pool.tile([128, 1024], mmdt, name=f"sp{j}",
                                 tag=f"sp{j}", bufs=1)
                off = KH * CK + BIAS + j * 1024
                nc.sync.dma_start(t[:], xw_d[:, off:off + 1024])
                sps.append(t)

            # quad0 streams as four k-slice DMAs so phase 2's first wave isn't
            # gated on the whole 1MB quad; quads 1..3 stay whole.
            w2qs = []
            q0 = w2_pool.tile([128, 4 * HIDDEN], mmdt, name="w2q0",
                              tag="w2q0", bufs=1)
            for kk in range(4):
                nc.sync.dma_start(q0[:, kk * HIDDEN:(kk + 1) * HIDDEN],
                                  w2s_d[:, kk * HIDDEN:(kk + 1) * HIDDEN])
            w2qs.append(q0)
            for q in range(1, 4):
                t = w2_pool.tile([128, 4 * HIDDEN], mmdt, name=f"w2q{q}",
                                 tag=f"w2q{q}", bufs=1)
                nc.sync.dma_start(
                    t[:], w2s_d[:, q * 4 * HIDDEN:(q + 1) * 4 * HIDDEN])
                w2qs.append(t)

            # PE p-state warmup: the PE runs below max clock until ~3us of
            # continuous busy. Burn the DMA-wait window with tiny matmuls on
            # a zeroed tile so the real stream starts warm.
            wz = wz_pool.tile([128, 128], mmdt, name="wz", tag="wz")
            nc.gpsimd.memset(wz[:], 0.0)
            wps = ps_pool.tile([128, 128], fp32, name="wps", tag="ps")
            for i in range(N_WARM):
                nc.tensor.matmul(wps[:], wz[:], wz[:], start=True, stop=True)

            o6b = out_pool.tile([128, 128], mmdt, name="os6b", tag="os6b")
            o7b = out_pool.tile([128, 128], mmdt, name="os7b", tag="os7b")

            # ---- phase 1, group A (chains 0..7): k-outer sweep ----
            hts = [None] * KF
            psA = [ps_pool.tile([128, cap], fp32, name=f"ps1_{m}", tag="ps")
                   for m in range(KH)]
            for k in range(KH):
                xk = cks[k][:, :cap]
                for m in range(KH):
                    nc.tensor.matmul(
                        psA[m][:], w1a_lhsT(k, m),
                        xk, start=(k == 0), stop=(k == KH - 1))
            for m in range(KH):
                ht = ht_pool.tile([128, cap], mmdt, name=f"ht{m}", tag="ht")
                nc.scalar.activation(
                    ht[:], psA[m][:], mybir.ActivationFunctionType.Gelu,
                    bias=b1s[:, m:m + 1])
                hts[m] = ht

            # ---- phase 1, chains 8..15: chain-outer against W1 strips ----
            for m in range(KH, KF):
                psb = ps_pool.tile([128, cap], fp32, name=f"ps1_{m}", tag="ps")
                for k in range(KH):
                    lhsT = sps[m - KH][:, k * 128:(k + 1) * 128]
                    nc.tensor.matmul(psb[:], lhsT, cks[k][:, :cap],
                                     start=(k == 0), stop=(k == KH - 1))
                ht = ht_pool.tile([128, cap], mmdt, name=f"ht{m}", tag="ht")
                nc.scalar.activation(
                    ht[:], psb[:], mybir.ActivationFunctionType.Gelu,
                    bias=b1s[:, m:m + 1])
                hts[m] = ht

            # ---- phase 2: yT[m] = W2.T @ hT + b2  [H on partitions] ----
            def w2_lhsT(k, m):
                q, kk = divmod(k, 4)
                off = kk * HIDDEN + m * 128
                return w2qs[q][:, off:off + 128]

            # chain order: full chains 0..5 first (their stops happen right
            # after the last W2 quad lands, so their pair stores hide under
            # the remaining chains), then the [0:SPL) halves of chains 7 and
            # 6 (one pair store), then the [SPL:cap) halves of 6 and 7 LAST,
            # exiting through the prepared scatters above — the post-last-
            # matmul tail is just evac + trigger + transfer + sem.
            # chains 0..3: one quad store (rows 0..511) fired on m3's evac
            ot4 = out_pool.tile([128, 4 * cap], mmdt, name="ot4", tag="ot4")
            for m in range(4):
                ps2 = ps_pool.tile([128, cap], fp32, name=f"ps2_{m}", tag="ps")
                for k in range(KF):
                    nc.tensor.matmul(ps2[:], w2_lhsT(k, m), hts[k][:],
                                     start=(k == 0), stop=(k == KF - 1))
                blk = ot4[:, m * cap:(m + 1) * cap]
                if m % 2 == 0:
                    nc.vector.tensor_scalar_add(blk, ps2[:], b2s[:, m:m + 1])
                else:
                    nc.scalar.activation(
                        blk, ps2[:], mybir.ActivationFunctionType.Identity,
                        bias=b2s[:, m:m + 1])
            nc.sync.dma_start(
                y16[0:512, :].rearrange("(c p) t -> p c t", p=128),
                ot4.rearrange("p (c t) -> p c t", c=4))

            # chains 4,5: pair store on the ACT ring
            ot2 = out_pool.tile([128, 2 * cap], mmdt, name="ot2", tag="ot2")
            for m in (4, 5):
                ps2 = ps_pool.tile([128, cap], fp32, name=f"ps2_{m}", tag="ps")
                for k in range(KF):
                    nc.tensor.matmul(ps2[:], w2_lhsT(k, m), hts[k][:],
                                     start=(k == 0), stop=(k == KF - 1))
                blk = ot2[:, (m - 4) * cap:(m - 3) * cap]
                if m % 2 == 0:
                    nc.vector.tensor_scalar_add(blk, ps2[:], b2s[:, m:m + 1])
                else:
                    nc.scalar.activation(
                        blk, ps2[:], mybir.ActivationFunctionType.Identity,
                        bias=b2s[:, m:m + 1])
            nc.scalar.dma_start(
                y16[512:768, :].rearrange("(c p) t -> p c t", p=128),
                ot2.rearrange("p (c t) -> p c t", c=2))

            # [0:SPL) halves of chains 7 then 6, one paired store on SP
            ota = out_pool.tile([128, 2 * SPL], mmdt, name="ota", tag="ota")
            for i, m in enumerate((KH - 1, KH - 2)):
                psx = ps_pool.tile([128, SPL], fp32, name=f"ps2_{m}a",
                                   tag="ps")
                for k in range(KF):
                    nc.tensor.matmul(psx[:], w2_lhsT(k, m), hts[k][:, :SPL],
                                     start=(k == 0), stop=(k == KF - 1))
                if i == 0:  # chain 7 half -> second column block
                    nc.scalar.activation(
                        ota[:, SPL:], psx[:],
                        mybir.ActivationFunctionType.Identity,
                        bias=b2s[:, m:m + 1])
                else:       # chain 6 half -> first column block, then store
                    nc.vector.tensor_scalar_add(ota[:, :SPL], psx[:],
                                                b2s[:, m:m + 1])
                    nc.sync.dma_start(
                        y16[(KH - 2) * 128:KH * 128, :SPL]
                        .rearrange("(c p) t -> p c t", p=128),
                        ota.rearrange("p (c t) -> p c t", c=2))

            # [SPL:cap) halves of chains 6 then 7 run LAST — narrow stores on
            # separate queues keep the post-last-matmul tail short
            for m, ob in ((KH - 2, o6b), (KH - 1, o7b)):
                psx = ps_pool.tile([128, 128], fp32, name=f"ps2_{m}b",
                                   tag="ps")
                for k in range(KF):
                    nc.tensor.matmul(psx[:], w2_lhsT(k, m), hts[k][:, SPL:],
                                     start=(k == 0), stop=(k == KF - 1))
                nc.vector.tensor_scalar_add(ob[:], psx[:], b2s[:, m:m + 1])
                eng = nc.scalar if m == KH - 2 else nc.sync
                eng.dma_start(y16[m * 128:(m + 1) * 128, SPL:], ob[:])

    _wire_prep_dma_sems(nc)
    _split_multi_waits(nc)
    return nc


def _get_nc(cap: int) -> bass.Bass:
    key = (cap, MM_DTYPE, N_WARM)
    if key not in _compiled_cache:
        _compiled_cache[key] = _build_nc(cap, MM_DTYPE)
    return _compiled_cache[key]


def _reference_numpy(x, idx, W1, b1, W2, b2):
    """Exact CPU path (erf-gelu in float64). Used only if routing is so
    imbalanced that one expert exceeds 512 tokens (breaks the device tiling)
    or the device path fails — slow but correct."""
    import math
    erf = np.vectorize(math.erf, otypes=[np.float64])
    out = np.zeros_like(x, dtype=np.float64)
    for e in range(NUM_EXPERTS):
        rows = np.nonzero(idx == e)[0]
        if rows.size == 0:
            continue
        h = x[rows].astype(np.float64) @ W1[e].astype(np.float64) + b1[e]
        h = h * 0.5 * (1.0 + erf(h / np.sqrt(2.0)))
        out[rows] = h @ W2[e].astype(np.float64) + b2[e]
    return out.astype(np.float32)


def kernel(x, expert_indices, W1, b1, W2, b2):
    x = np.ascontiguousarray(np.asarray(x, dtype=np.float32))
    idx = np.asarray(expert_indices).astype(np.int64)
    W1 = np.asarray(W1, dtype=np.float32)
    W2 = np.asarray(W2, dtype=np.float32)
    b1 = np.asarray(b1, dtype=np.float32)
    b2 = np.asarray(b2, dtype=np.float32)

    counts = np.bincount(idx, minlength=NUM_EXPERTS)
    # one PSUM bank caps the per-chain moving dim at 512 fp32
    cap = max(256, int(-(-int(counts.max()) // 2)) * 2)
    if cap > 512:  # pathological routing, exceeds one PSUM bank
        return _reference_numpy(x, idx, W1, b1, W2, b2)
    nc = _get_nc(cap)

    # dispatch: stable sort tokens by expert
    order = np.argsort(idx, kind="stable")
    starts = np.zeros(NUM_EXPERTS + 1, dtype=np.int64)
    np.cumsum(counts, out=starts[1:])

    np_mmdt = np.float16 if MM_DTYPE == mybir.dt.float16 else np.float32
    CK = cap + HIDDEN
    in_maps = []
    tok_of_core = []
    for e in range(NUM_EXPERTS):
        toks = order[starts[e]:starts[e + 1]]
        tok_of_core.append(toks)
        xs = np.zeros((KH, 128, cap), dtype=np_mmdt)
        xs.reshape(HIDDEN, cap)[:, :len(toks)] = x[toks].T
        w1e = W1[e].astype(np_mmdt)
        w1rows = w1e.reshape(KH, 128, FFN)
        biasv = np.concatenate([
            np.ascontiguousarray(b1[e].reshape(KF, 128).T),
            np.ascontiguousarray(b2[e].reshape(KH, 128).T),
        ], axis=1).view(np_mmdt)  # fp32 biases as raw lanes of stream dtype
        chunks = []
        for k in range(KH):
            parts = [xs[k], w1rows[k][:, :HIDDEN]]
            if k == KH - 1:
                parts.append(biasv)
            chunks.append(np.concatenate(parts, axis=1))
        strips = w1e[:, HIDDEN:].reshape(KH, 128, KH, 128) \
            .transpose(1, 2, 0, 3).reshape(128, -1)  # [128, 8 strips of 1024]
        xw = np.concatenate(chunks + [strips], axis=1)
        w2s = W2[e].astype(np_mmdt).reshape(KF, 128, HIDDEN) \
            .transpose(1, 0, 2).reshape(128, -1)
        in_maps.append({
            "xw": np.ascontiguousarray(xw),
            "w2s": np.ascontiguousarray(w2s),
        })

    try:
        res = _run_spmd_cached(nc, in_maps)
    except Exception:
        try:  # transient failures recover on retry; fall back to the shim
            res = run_bass_kernel_spmd(nc, in_maps,
                                       core_ids=list(range(N_CORES)))
        except Exception:
            return _reference_numpy(x, idx, W1, b1, W2, b2)
    global LAST_RESULTS
    LAST_RESULTS = res

    out = np.zeros((TOKENS, HIDDEN), dtype=np.float32)
    for e in range(NUM_EXPERTS):
        toks = tok_of_core[e]
        yT = res.results[e]["y16"].astype(np.float32)
        out[toks] = yT[:, :len(toks)].T
    return out


# revision 52
# speedup vs baseline: 1.0142x; 1.0054x over previous
"""MoE expert-parallel kernel for Trainium2 (8 NeuronCores).

Problem: nn_DistributedExpertPool — each of 2048 tokens (H=1024) is routed to
one of 8 experts; expert e applies Linear(H->F=2048) -> exact GELU ->
Linear(F->H).

Strategy (expert parallelism, matching the sharding hint):
  - Host: sort tokens by expert assignment ("dispatch"), pad each expert's
    token batch to a common capacity CAP, and pre-transpose to x.T layout so
    the device kernel only ever streams K-major operands.
  - Core c gets expert c's weights plus its token batch, computes
    y.T = W2.T @ gelu(W1.T @ x.T + b1) + b2 entirely on-chip.
  - Host: scatter each core's outputs back to the original token order
    ("combine").

Schedule notes (all sizes for cap~274):
  - Phase 1 opens with a k-outer sweep over output chains 0..7 so the first
    matmuls only need chunk0 = [x_k0 | W1 k-row 0 of chains 0..7] (~330KB)
    instead of all of x + a full W1 strip (~820KB). Chunks k=1..7 stream one
    per k-step, each arriving just ahead of the PE.
  - Chains 8..15 then run chain-outer against per-chain W1 strips (their PSUM
    banks free up as the group-A GELUs drain).
  - Phase 2 is chain-outer over 8 H-chains against a k-major W2 stream; the
    Tile scheduler interleaves chains so the PE never waits on a W2 quad.
  - The last H-chain is split column-wise so the final store is a narrow
    [128,128] tile — shortens the post-matmul tail.
  - b1/b2 ride as extra fp16-bitcast columns on chunk7 (no extra DMA).
  - Evacs/stores are fp16 (output rounding ~5e-4 relative, tolerance 2e-2).

Matmul operands stream as fp16 (weights ~N(0, 0.02), activations O(1) — well
inside fp16 range). PSUM accumulation stays fp32. Measured ~4e-4 relative
error end-to-end vs the fp32 reference.
"""

import os as _os
import sys as _sys

import numpy as np

try:
    import concourse.bass as bass
except ImportError:  # fresh dirs without the site hook on sys.path
    for _p in ("/opt/trn_rl_repo", "/root/.axon_site/_ro/trn_rl_repo"):
        if _p not in _sys.path:
            _sys.path.append(_p)
    import concourse.bass as bass  # noqa: E402
import concourse.tile as tile
from concourse import mybir
from concourse.bass_utils import run_bass_kernel_spmd  # noqa: F401 (fallback)

_jit_cache: dict[int, tuple] = {}


def _run_spmd_cached(nc, in_maps):
    """run_bass_kernel_spmd's axon/PJRT path with the jitted executable cached
    per program — the concourse shim rebuilds its jax.jit closure every call,
    paying ~1.5s of retrace; reusing one function object makes repeat calls
    dispatch in milliseconds."""
    import jax
    import numpy as _np
    from jax.sharding import Mesh, PartitionSpec
    from jax.experimental.shard_map import shard_map
    from concourse import bass2jax, mybir as _mb

    key = id(nc)
    if key not in _jit_cache:
        bass2jax.install_neuronx_cc_hook()
        partition_name = (nc.partition_id_tensor.name
                          if nc.partition_id_tensor else None)
        in_names, out_names, out_avals = [], [], []
        for alloc in nc.m.functions[0].allocations:
            if not isinstance(alloc, _mb.MemoryLocationSet):
                continue
            name = alloc.memorylocations[0].name
            if alloc.kind == "ExternalInput":
                if name != partition_name:
                    in_names.append(name)
            elif alloc.kind == "ExternalOutput":
                out_names.append(name)
                out_avals.append(jax.core.ShapedArray(
                    tuple(alloc.tensor_shape), _mb.dt.np(alloc.dtype)))
        n_params = len(in_names)
        all_names = list(in_names) + list(out_names)
        if partition_name is not None:
            all_names.append(partition_name)

        def _body(*args):
            operands = list(args)
            if partition_name is not None:
                operands.append(bass2jax.partition_id_tensor())
            return tuple(bass2jax._bass_exec_p.bind(
                *operands, out_avals=tuple(out_avals),
                in_names=tuple(all_names), out_names=tuple(out_names),
                lowering_input_output_aliases=(),
                sim_require_finite=True, sim_require_nnan=True, nc=nc))

        devices = jax.devices()[:N_CORES]
        mesh = Mesh(_np.asarray(devices), ("core",))
        n_outs = len(out_names)
        sharded = jax.jit(
            shard_map(_body, mesh=mesh,
                      in_specs=(PartitionSpec("core"),) * (n_params + n_outs),
                      out_specs=(PartitionSpec("core"),) * n_outs,
                      check_rep=False),
            donate_argnums=tuple(range(n_params, n_params + n_outs)),
            keep_unused=True)
        _jit_cache[key] = (sharded, in_names, out_names, out_avals, n_params)

    sharded, in_names, out_names, out_avals, n_params = _jit_cache[key]
    concat_in = [
        _np.concatenate([_np.asarray(m[name]) for m in in_maps], axis=0)
        for name in in_names]
    concat_zeros = [
        _np.zeros((N_CORES * a.shape[0], *a.shape[1:]), a.dtype)
        for a in out_avals]
    out_arrs = sharded(*concat_in, *concat_zeros)

    class _R:
        results = [
            {name: _np.asarray(out_arrs[i]).reshape(
                N_CORES, *out_avals[i].shape)[c]
             for i, name in enumerate(out_names)}
            for c in range(N_CORES)]
    return _R()

TOKENS = 2048
HIDDEN = 1024
FFN = 2048
NUM_EXPERTS = 8
N_CORES = 8

KH = HIDDEN // 128  # 8 K-tiles for the first matmul
KF = FFN // 128     # 16 K-tiles for the second matmul

_compiled_cache: dict[tuple, bass.Bass] = {}

MM_DTYPE = {"fp32": mybir.dt.float32, "fp32r": mybir.dt.float32r,
            "fp16": mybir.dt.float16}[_os.environ.get("KM_MMDT", "fp16")]
N_WARM = int(_os.environ.get("KM_WARM", "26"))


def _split_multi_waits(nc: bass.Bass) -> None:
    """Walrus in this toolchain accepts at most ONE sync-wait per instruction
    ("Too many sync wait commands" in setupSyncWait otherwise). Tile's
    scheduler happily attaches several. Split the extras into NoOps placed
    just before the instruction on the same engine queue — the NX sequencer
    processes them in order, so the semantics are identical."""
    for fn in nc.m.functions:
        # program position of each semaphore's final updater — waits that
        # resolve latest go LAST in the split so the already-satisfied NoOps
        # are off the final drain's critical path
        last_upd: dict[int, int] = {}
        pos = 0
        for blk in fn.blocks:
            for inst in blk.instructions:
                pos += 1
                si = inst.sync_info
                if si and si.on_update:
                    for u in si.on_update:
                        if u.sync_type == "semaphore":
                            last_upd[u.id] = pos
        for blk in fn.blocks:
            out = []
            changed = False
            for inst in blk.instructions:
                si = inst.sync_info
                if si is not None and si.on_wait is not None and len(si.on_wait) > 1:
                    waits = sorted(si.on_wait,
                                   key=lambda w: last_upd.get(w.id, 0))
                    for j, w in enumerate(waits[:-1]):
                        nop = mybir.InstNoOp(
                            name=f"{inst.name}-wsplit{j}", ins=[], outs=[])
                        nop.engine = inst.engine
                        nop.sync_info = mybir.SyncInfo(on_wait=[w], on_update=[])
                        out.append(nop)
                    inst.sync_info = mybir.SyncInfo(
                        on_wait=[waits[-1]],
                        on_update=list(si.on_update) if si.on_update else [],
                    )
                    changed = True
                out.append(inst)
            if changed:
                blk.instructions = out


def _wire_prep_dma_sems(nc: bass.Bass) -> None:
    """Point each PREPARE_ONLY prep's DMA-completion sem (on_update[0]) at the
    DMASW lane semaphore Tile's epilogue actually waits on. Tile attributes
    the deferred DMA write to the prep's DMASW tick, but leaves the
    user-supplied sem= in on_update[0] — so the lane sem would never move and
    the end-of-kernel drain deadlocks. Rewiring the update target makes the
    descriptor-baked sem BE the lane sem."""
    for fn in nc.m.functions:
        updated: set[int] = set()
        waited: dict[int, str] = {}
        insts = [i for blk in fn.blocks for i in blk.instructions]
        for inst in insts:
            si = inst.sync_info
            if si is None:
                continue
            for u in (si.on_update or []):
                if u.sync_type == "semaphore":
                    updated.add(u.id)
            for w in (si.on_wait or []):
                if w.sync_type == "semaphore" and str(w.ant_name or "").startswith("DMASW"):
                    waited[w.id] = w.ant_name
        dangling = [(i, n) for i, n in sorted(waited.items()) if i not in updated]
        if not dangling:
            continue
        # Tile assigns DMASW lanes to Pool-engine DMA instructions round-robin
        # in program order; mirror that to pair each prep with its lane.
        lanes = sorted(dangling, key=lambda t: t[1])
        preps = [i for i in insts if getattr(i, "gen_mode", 0) == 1]
        for pi, prep in enumerate(preps):
            sem_id, sem_name = lanes[pi % len(lanes)]
            u0 = prep.sync_info.on_update[0]
            u0.id = sem_id
            u0.ant_name = sem_name


def _build_nc(cap: int, mm_dtype=None) -> bass.Bass:
    """Build the per-core Bass program for token capacity `cap` (mult of 2)."""
    fp32 = mybir.dt.float32
    mmdt = MM_DTYPE if mm_dtype is None else mm_dtype
    nc = bass.Bass("TRN2", target_bir_lowering=False, debug=False,
                   num_devices=N_CORES)

    CK = cap + KH * 128            # chunk width: x k-block + W1 k-row of A
    # biases ride on chunk7 as raw lanes of the stream dtype; fp32 biases
    # need 2 fp16 lanes per value
    LW = 4 // mybir.dt.size(mmdt)  # lanes per fp32 value
    BIAS = LW * (KF + KH)
    # xw: 8 chunks (last one carries biases) + strips for chains 8..15
    xw_d = nc.dram_tensor("xw", [128, KH * CK + BIAS + KH * 1024], mmdt,
                          kind="ExternalInput").ap()
    # w2s: k-major quads [k-slice 0..3 | 4..7 | ...], k-slice = [128, HIDDEN]
    w2s_d = nc.dram_tensor("w2s", [128, KF * HIDDEN], mmdt,
                           kind="ExternalInput").ap()
    y16 = nc.dram_tensor("y16", [HIDDEN, cap], mmdt, kind="ExternalOutput").ap()

    SPL = cap - 128  # last two H-chains split into [0:SPL) and [SPL:cap)

    with tile.TileContext(nc) as tc:
        with (
            tc.tile_pool(name="ck_pool", bufs=KH) as ck_pool,
            tc.tile_pool(name="w1_pool", bufs=KH) as w1_pool,
            tc.tile_pool(name="w2_pool", bufs=1) as w2_pool,
            tc.tile_pool(name="wz_pool", bufs=1) as wz_pool,
            tc.tile_pool(name="ht_pool", bufs=KF) as ht_pool,
            tc.tile_pool(name="out_pool", bufs=4) as out_pool,
            tc.tile_pool(name="ps_pool", bufs=8, space="PSUM") as ps_pool,
        ):
            # --- load stream (SP queue): chunks 0..7, strips, quads ---
            cks = []
            for k in range(KH):
                w = CK + (BIAS if k == KH - 1 else 0)
                t = ck_pool.tile([128, w], mmdt, name=f"ck{k}", tag=f"ck{k}",
                                 bufs=1)
                off = k * CK
                nc.sync.dma_start(t[:], xw_d[:, off:off + w])
                cks.append(t)
            b1s = cks[7][:, CK:CK + LW * KF].bitcast(fp32)      # [128, KF]
            b2s = cks[7][:, CK + LW * KF:CK + BIAS].bitcast(fp32)  # [128, KH]

            def w1a_lhsT(k, m):
                return cks[k][:, cap + m * 128:cap + (m + 1) * 128]

            sps = []
            for j in range(KH):
                t = w1_pool.tile([128, 1024], mmdt, name=f"sp{j}",
                                 tag=f"sp{j}", bufs=1)
                off = KH * CK + BIAS + j * 1024
                nc.sync.dma_start(t[:], xw_d[:, off:off + 1024])
                sps.append(t)

            # quad0 streams as four k-slice DMAs so phase 2's first wave isn't
            # gated on the whole 1MB quad; quads 1..3 stay whole.
            w2qs = []
            q0 = w2_pool.tile([128, 4 * HIDDEN], mmdt, name="w2q0",
                              tag="w2q0", bufs=1)
            for kk in range(4):
                nc.sync.dma_start(q0[:, kk * HIDDEN:(kk + 1) * HIDDEN],
                                  w2s_d[:, kk * HIDDEN:(kk + 1) * HIDDEN])
            w2qs.append(q0)
            for q in range(1, 4):
                t = w2_pool.tile([128, 4 * HIDDEN], mmdt, name=f"w2q{q}",
                                 tag=f"w2q{q}", bufs=1)
                nc.sync.dma_start(
                    t[:], w2s_d[:, q * 4 * HIDDEN:(q + 1) * 4 * HIDDEN])
                w2qs.append(t)

            # PE p-state warmup: the PE runs below max clock until ~3us of
            # continuous busy. Burn the DMA-wait window with tiny matmuls on
            # a zeroed tile so the real stream starts warm.
            wz = wz_pool.tile([128, 128], mmdt, name="wz", tag="wz")
            nc.gpsimd.memset(wz[:], 0.0)
            wps = ps_pool.tile([128, 128], fp32, name="wps", tag="ps")
            for i in range(N_WARM):
                nc.tensor.matmul(wps[:], wz[:], wz[:], start=True, stop=True)

            o6b = out_pool.tile([128, 128], mmdt, name="os6b", tag="os6b")
            o7b = out_pool.tile([128, 128], mmdt, name="os7b", tag="os7b")

            # ---- phase 1, group A (chains 0..7): k-outer sweep ----
            hts = [None] * KF
            psA = [ps_pool.tile([128, cap], fp32, name=f"ps1_{m}", tag="ps")
                   for m in range(KH)]
            for k in range(KH):
                xk = cks[k][:, :cap]
                for m in range(KH):
                    nc.tensor.matmul(
                        psA[m][:], w1a_lhsT(k, m),
                        xk, start=(k == 0), stop=(k == KH - 1))
            for m in range(KH):
                ht = ht_pool.tile([128, cap], mmdt, name=f"ht{m}", tag="ht")
                nc.scalar.activation(
                    ht[:], psA[m][:], mybir.ActivationFunctionType.Gelu,
                    bias=b1s[:, m:m + 1])
                hts[m] = ht

            # ---- phase 1, chains 8..15: chain-outer against W1 strips ----
            for m in range(KH, KF):
                psb = ps_pool.tile([128, cap], fp32, name=f"ps1_{m}", tag="ps")
                for k in range(KH):
                    lhsT = sps[m - KH][:, k * 128:(k + 1) * 128]
                    nc.tensor.matmul(psb[:], lhsT, cks[k][:, :cap],
                                     start=(k == 0), stop=(k == KH - 1))
                ht = ht_pool.tile([128, cap], mmdt, name=f"ht{m}", tag="ht")
                nc.scalar.activation(
                    ht[:], psb[:], mybir.ActivationFunctionType.Gelu,
                    bias=b1s[:, m:m + 1])
                hts[m] = ht

            # ---- phase 2: yT[m] = W2.T @ hT + b2  [H on partitions] ----
            def w2_lhsT(k, m):
                q, kk = divmod(k, 4)
                off = kk * HIDDEN + m * 128
                return w2qs[q][:, off:off + 128]

            # chain order: full chains 0..5 first (their stops happen right
            # after the last W2 quad lands, so their pair stores hide under
            # the remaining chains), then the [0:SPL) halves of chains 7 and
            # 6 (one pair store), then the [SPL:cap) halves of 6 and 7 LAST,
            # exiting through the prepared scatters above — the post-last-
            # matmul tail is just evac + trigger + transfer + sem.
            # chains 0..3: one quad store (rows 0..511) fired on m3's evac
            ot4 = out_pool.tile([128, 4 * cap], mmdt, name="ot4", tag="ot4")
            for m in range(4):
                ps2 = ps_pool.tile([128, cap], fp32, name=f"ps2_{m}", tag="ps")
                for k in range(KF):
                    nc.tensor.matmul(ps2[:], w2_lhsT(k, m), hts[k][:],
                                     start=(k == 0), stop=(k == KF - 1))
                blk = ot4[:, m * cap:(m + 1) * cap]
                if m % 2 == 0:
                    nc.vector.tensor_scalar_add(blk, ps2[:], b2s[:, m:m + 1])
                else:
                    nc.scalar.activation(
                        blk, ps2[:], mybir.ActivationFunctionType.Identity,
                        bias=b2s[:, m:m + 1])
            nc.sync.dma_start(
                y16[0:512, :].rearrange("(c p) t -> p c t", p=128),
                ot4.rearrange("p (c t) -> p c t", c=4))

            # chains 4,5: pair store on the ACT ring
            ot2 = out_pool.tile([128, 2 * cap], mmdt, name="ot2", tag="ot2")
            for m in (4, 5):
                ps2 = ps_pool.tile([128, cap], fp32, name=f"ps2_{m}", tag="ps")
                for k in range(KF):
                    nc.tensor.matmul(ps2[:], w2_lhsT(k, m), hts[k][:],
                                     start=(k == 0), stop=(k == KF - 1))
                blk = ot2[:, (m - 4) * cap:(m - 3) * cap]
                if m % 2 == 0:
                    nc.vector.tensor_scalar_add(blk, ps2[:], b2s[:, m:m + 1])
                else:
                    nc.scalar.activation(
                        blk, ps2[:], mybir.ActivationFunctionType.Identity,
                        bias=b2s[:, m:m + 1])
            nc.scalar.dma_start(
                y16[512:768, :].rearrange("(c p) t -> p c t", p=128),
                ot2.rearrange("p (c t) -> p c t", c=2))

            # [0:SPL) halves of chains 7 then 6, one paired store on SP
            ota = out_pool.tile([128, 2 * SPL], mmdt, name="ota", tag="ota")
            for i, m in enumerate((KH - 1, KH - 2)):
                psx = ps_pool.tile([128, SPL], fp32, name=f"ps2_{m}a",
                                   tag="ps")
                for k in range(KF):
                    nc.tensor.matmul(psx[:], w2_lhsT(k, m), hts[k][:, :SPL],
                                     start=(k == 0), stop=(k == KF - 1))
                if i == 0:  # chain 7 half -> second column block
                    nc.scalar.activation(
                        ota[:, SPL:], psx[:],
                        mybir.ActivationFunctionType.Identity,
                        bias=b2s[:, m:m + 1])
                else:       # chain 6 half -> first column block, then store
                    nc.vector.tensor_scalar_add(ota[:, :SPL], psx[:],
                                                b2s[:, m:m + 1])
                    nc.sync.dma_start(
                        y16[(KH - 2) * 128:KH * 128, :SPL]
                        .rearrange("(c p) t -> p c t", p=128),
                        ota.rearrange("p (c t) -> p c t", c=2))

            # [SPL:cap) halves of chains 6 then 7 run LAST — narrow stores on
            # separate queues keep the post-last-matmul tail short
            for m, ob in ((KH - 2, o6b), (KH - 1, o7b)):
                psx = ps_pool.tile([128, 128], fp32, name=f"ps2_{m}b",
                                   tag="ps")
                for k in range(KF):
                    nc.tensor.matmul(psx[:], w2_lhsT(k, m), hts[k][:, SPL:],
                                     start=(k == 0), stop=(k == KF - 1))
                nc.vector.tensor_scalar_add(ob[:], psx[:], b2s[:, m:m + 1])
                eng = nc.scalar if m == KH - 2 else nc.sync
                eng.dma_start(y16[m * 128:(m + 1) * 128, SPL:], ob[:])

    _wire_prep_dma_sems(nc)
    _split_multi_waits(nc)
    return nc


def _get_nc(cap: int) -> bass.Bass:
    key = (cap, MM_DTYPE, N_WARM)
    if key not in _compiled_cache:
        _compiled_cache[key] = _build_nc(cap, MM_DTYPE)
    return _compiled_cache[key]


def _reference_numpy(x, idx, W1, b1, W2, b2):
    """Exact CPU path (erf-gelu in float64). Used only if routing is so
    imbalanced that one expert exceeds 512 tokens (breaks the device tiling)
    or the device path fails — slow but correct."""
    import math
    erf = np.vectorize(math.erf, otypes=[np.float64])
    out = np.zeros_like(x, dtype=np.float64)
    for e in range(NUM_EXPERTS):
        rows = np.nonzero(idx == e)[0]
        if rows.size == 0:
            continue
        h = x[rows].astype(np.float64) @ W1[e].astype(np.float64) + b1[e]
        h = h * 0.5 * (1.0 + erf(h / np.sqrt(2.0)))
        out[rows] = h @ W2[e].astype(np.float64) + b2[e]
    return out.astype(np.float32)


def kernel(x, expert_indices, W1, b1, W2, b2):
    x = np.ascontiguousarray(np.asarray(x, dtype=np.float32))
    idx = np.asarray(expert_indices).astype(np.int64)
    W1 = np.asarray(W1, dtype=np.float32)
    W2 = np.asarray(W2, dtype=np.float32)
    b1 = np.asarray(b1, dtype=np.float32)
    b2 = np.asarray(b2, dtype=np.float32)

    counts = np.bincount(idx, minlength=NUM_EXPERTS)
    # one PSUM bank caps the per-chain moving dim at 512 fp32
    cap = max(256, int(-(-int(counts.max()) // 2)) * 2)
    if cap > 512:  # pathological routing, exceeds one PSUM bank
        return _reference_numpy(x, idx, W1, b1, W2, b2)
    nc = _get_nc(cap)

    # dispatch: stable sort tokens by expert
    order = np.argsort(idx, kind="stable")
    starts = np.zeros(NUM_EXPERTS + 1, dtype=np.int64)
    np.cumsum(counts, out=starts[1:])

    np_mmdt = np.float16 if MM_DTYPE == mybir.dt.float16 else np.float32
    CK = cap + HIDDEN
    in_maps = []
    tok_of_core = []
    for e in range(NUM_EXPERTS):
        toks = order[starts[e]:starts[e + 1]]
        tok_of_core.append(toks)
        xs = np.zeros((KH, 128, cap), dtype=np_mmdt)
        xs.reshape(HIDDEN, cap)[:, :len(toks)] = x[toks].T
        w1e = W1[e].astype(np_mmdt)
        w1rows = w1e.reshape(KH, 128, FFN)
        biasv = np.concatenate([
            np.ascontiguousarray(b1[e].reshape(KF, 128).T),
            np.ascontiguousarray(b2[e].reshape(KH, 128).T),
        ], axis=1).view(np_mmdt)  # fp32 biases as raw lanes of stream dtype
        chunks = []
        for k in range(KH):
            parts = [xs[k], w1rows[k][:, :HIDDEN]]
            if k == KH - 1:
                parts.append(biasv)
            chunks.append(np.concatenate(parts, axis=1))
        strips = w1e[:, HIDDEN:].reshape(KH, 128, KH, 128) \
            .transpose(1, 2, 0, 3).reshape(128, -1)  # [128, 8 strips of 1024]
        xw = np.concatenate(chunks + [strips], axis=1)
        w2s = W2[e].astype(np_mmdt).reshape(KF, 128, HIDDEN) \
            .transpose(1, 0, 2).reshape(128, -1)
        in_maps.append({
            "xw": np.ascontiguousarray(xw),
            "w2s": np.ascontiguousarray(w2s),
        })

    try:
        res = _run_spmd_cached(nc, in_maps)
    except Exception:
        try:  # transient failures recover on retry; fall back to the shim
            res = run_bass_kernel_spmd(nc, in_maps,
                                       core_ids=list(range(N_CORES)))
        except Exception:
            return _reference_numpy(x, idx, W1, b1, W2, b2)
    global LAST_RESULTS
    LAST_RESULTS = res

    out = np.zeros((TOKENS, HIDDEN), dtype=np.float32)
    for e in range(NUM_EXPERTS):
        toks = tok_of_core[e]
        yT = res.results[e]["y16"].astype(np.float32)
        out[toks] = yT[:, :len(toks)].T
    return out
